# revision 48
# baseline (speedup 1.0000x reference)
"""Trainium2 Bass kernel for 4-head spatial self-attention (f16 pipeline).

Computation (per batch b):
    xf = x[b] reshaped [C=256, n=4096]
    q/k/v = Wq/Wk/Wv @ xf            -> [128, n]   (rows = 4 heads x 32 dims)
    S_h   = (q_h^T k_h) * 32^-0.5    -> [n, n] per head
    P     = exp(S)   (softmax without max-subtraction: logits are O(6))
    A_h   = P_h^T-normalized @ v_h   -> [n, 32]
    out   = Wout @ A + bout          -> [C, n]

Sharding: 8 cores = 4 batches x 2 query-halves. Each core handles all 4 heads
for one batch and 2048 queries vs all 4096 keys; outputs are disjoint slices.

Design notes (cost-model driven):
 - All matmuls run in f16 (1 PE cycle/output-column vs 4 for fp32).
 - S is computed TRANSPOSED (keys on partitions, queries free), 4 heads packed
   onto PE row strips via tile_position (32h, 0); each head's [128, 512] S^T
   needs its own PSUM bank (probed HW constraint for concurrent row strips),
   so heads go in pairs to 2-bank tiles [128, 2, 512].
 - exp is split across TWO engines, one head-pair tile each per key chunk:
   ScalarE computes exact exp -> f16; DVE computes a Schraudolph
   approximation (round(S*A+B) as int16 IS the f16 bit pattern of
   exp(S*SCALE), B tuned on-device). Softmax renormalization absorbs the
   ~2% approximation noise.
 - PV runs TRANSPOSED as well: A^T[q,d] = sum_j P^T[j,q]^T v^T[j,d], with the
   512-wide P^T chunk as the STATIONARY operand and the 33-wide v chunk as
   the MOVING operand, accumulating over the 32 key chunks. vT carries an
   extra ones column so A^T column 32 is the softmax denominator -- a
   per-partition scalar, normalized with one reciprocal + broadcast multiply.
   Probed HW constraint: only one OPEN accumulation group per PSUM bank, so
   the 16 groups (4 q-subchunks x 4 heads) run as a sequential tail per
   query block, software-pipelined against the next block's S^T/exp stream
   (the P^T tiles of a block stay resident in SBUF: 64 tiles + slack).
 - an^T -> an via one batched DMA-transpose (16x128 xbar tiles, f16), then
   a plain [c,q] = Wout^T.T @ an out-projection + bias, DMA'd out per
   query block. The first query block carries the q/k/v projections JIT
   (x column-blocks staged in recyclable P^T-pool slots); the final tail
   pipelines bank A's finishers against bank B's PV^T and uses a PE
   transpose (vs DMA) on the critical path. A few warm-up matmuls at t=0
   keep the tensor-engine clock ramped before the first projections.
"""

import numpy as np
import sys

for _p in ("/opt/trn_rl_repo", "/opt/pypackages"):
    if _p not in sys.path:
        sys.path.append(_p)

import concourse.bass as bass
import concourse.tile as tile
from concourse import bacc, mybir
from concourse.tile import add_dep_helper
from concourse.bass_utils import run_bass_kernel_spmd

f32 = mybir.dt.float32
f16 = mybir.dt.float16
i16 = mybir.dt.int16

B = 4
C = 256
N = 4096          # h*w = 64*64 key positions
NQ = 2048         # queries per core (half batch)
HEADS = 4
DH = 32
INNER = 128
SCALE = DH ** -0.5

QB = 512          # query block (free dim of S^T tiles)
NQB = NQ // QB    # 4
JT = 128          # key tile (partition dim of S^T tiles)
NJT = N // JT     # 32

PT_BUFS = 86   # P^T slots: 64 resident + next-block growth + xkv staging

# Schraudolph f16 exp: int16(round(S*A_EXP + B_EXP)) bitcast to f16
A_EXP = SCALE * 1024.0 / float(np.log(2.0))
B_EXP = 15365.0

# ScalarE : DVE exp split. qb>0: strict alternation (p0 -> ScalarE,
# p1 -> DVE). qb0 carries the JIT projection copies (k copies on ScalarE,
# q/v copies on DVE), so its table routes exp tiles away from whichever
# engine owns that J's copy, with a couple of extra ScalarE J's to offset
# DVE's larger copy load.
def _qb0_act(J, p):
    return p == 0


# qb>0: mostly strict alternation, but every 8th J sends both tiles to
# ScalarE -- ScalarE is ~13% faster per tile, so this keeps long-run balance
# while minimizing the Schraudolph (DVE) share for accuracy.
def _act_assign(qb, J, p):
    if qb == 0:
        return _qb0_act(J, p)
    return p == 0


def build_nc():
    nc = bacc.Bacc()

    xkv_d = nc.dram_tensor("xkv", [C, N], f16, kind="ExternalInput")
    wqT_d = nc.dram_tensor("wqT", [C, INNER], f16, kind="ExternalInput")
    wkT_d = nc.dram_tensor("wkT", [C, INNER], f16, kind="ExternalInput")
    wvT_d = nc.dram_tensor("wvT", [C, INNER], f16, kind="ExternalInput")
    woT_d = nc.dram_tensor("woT", [INNER, C], f16, kind="ExternalInput")
    biasT_d = nc.dram_tensor("biasT", [128, 2], f32, kind="ExternalInput")
    eye_d = nc.dram_tensor("eye", [128, 128], f16, kind="ExternalInput")
    out_d = nc.dram_tensor("out", [C, NQ], f32, kind="ExternalOutput")

    # One program for all 8 cores: the host passes xkv ROLLED so this core's
    # queries sit in columns 0:NQ. Key order is shared by k and v (both come
    # from the same rolled xkv), and softmax sums are order-invariant.
    q0 = 0

    with tile.TileContext(nc) as tc:
        import contextlib

        ctx = contextlib.ExitStack()
        with ctx:
            big = ctx.enter_context(tc.tile_pool(name="big", bufs=1))
            wk = ctx.enter_context(tc.tile_pool(name="wk", bufs=2))
            ptp = ctx.enter_context(tc.tile_pool(name="ptp", bufs=PT_BUFS))
            ps_st = ctx.enter_context(tc.tile_pool(name="ps_st", bufs=3, space="PSUM"))

            # ---- constants / weights ----
            wqT_sb = big.tile([128, 2, INNER], f16)   # [c_part, c_chunk, inner]
            wkT_sb = big.tile([128, 2, INNER], f16)
            wvT_sb = big.tile([128, 2, INNER], f16)
            woT_sb = big.tile([128, C], f16)          # [inner, c]
            bias_sb = big.tile([128, 2], f32)
            wqT_v = wqT_d.rearrange("(cc p) i -> p cc i", cc=2)
            wkT_v = wkT_d.rearrange("(cc p) i -> p cc i", cc=2)
            wvT_v = wvT_d.rearrange("(cc p) i -> p cc i", cc=2)
            xkv_v = xkv_d.rearrange("(cc p) n -> p cc n", cc=2)
            # ---- activations in: xkv lives in recyclable pt-pool slots ----
            # column-block tiles [128, 2(cc), 512]; slot is recycled into the
            # P^T pool once the projections for that block are done.
            nc.sync.dma_start(out=wqT_sb[:], in_=wqT_v)
            xkvt = []
            for t in range(N // 512):
                xt = ptp.tile([128, 2, 512], f16, tag="pt", name="xkvt")
                if t == 0:
                    for cc in range(2):
                        nc.sync.dma_start(
                            out=xt[:, cc, :],
                            in_=xkv_v[:, cc, 0:512],
                        )
                    nc.sync.dma_start(out=wkT_sb[:], in_=wkT_v)
                else:
                    nc.sync.dma_start(out=xt[:], in_=xkv_v[:, :, 512 * t:512 * (t + 1)])
                xkvt.append(xt)
                if t == 1:
                    nc.sync.dma_start(out=wvT_sb[:], in_=wvT_v)
            nc.sync.dma_start(out=woT_sb[:], in_=woT_d[:])
            nc.sync.dma_start(out=bias_sb[:], in_=biasT_d[:])
            eye_sb = big.tile([128, 128], f16)
            nc.sync.dma_start(out=eye_sb[:], in_=eye_d[:])

            # PE warm-up: the cost model treats the tensor engine as warm
            # only after ~3us of activity (or at t=0); a few dummy matmuls on
            # garbage SBUF keep the clock ramped before the first real ones.
            dummy_sb = big.tile([128, 512], f16)
            nc.gpsimd.memset(dummy_sb[:], 1.0)
            warm = ps_st.tile([128, 512], f32, tag="st", name="warm")
            for _ in range(8):
                nc.tensor.matmul(
                    out=warm[:], lhsT=dummy_sb[:, 0:128], rhs=dummy_sb[:],
                    start=True, stop=True, skip_group_check=True,
                )

            k_sb = big.tile([128, N], f16)     # [inner, n]
            q_sb = big.tile([128, NQ], f16)    # [inner, nq]
            # v^T chunks + ones col: [j0, (jtile, head), 33]; col 32 = 1.0
            vT3 = big.tile([128, NJT * HEADS, DH + 1], f16)
            nc.gpsimd.memset(vT3[:, :, 32:33], 1.0)

            # ---- projections (emitted JIT inside qb0's J-loop) ----
            def emit_kproj(t):
                kp = ps_st.tile([128, 512], f32, tag="acc", bufs=1, name="kp")
                for cc in range(2):
                    nc.tensor.matmul(
                        out=kp[:],
                        lhsT=wkT_sb[:, cc, :],
                        rhs=xkvt[t][:, cc, :],
                        start=(cc == 0), stop=(cc == 1),
                    )
                nc.scalar.copy(out=k_sb[:, 512 * t:512 * (t + 1)], in_=kp[:])

            def emit_qproj(t):
                qp = ps_st.tile([128, 512], f32, tag="acc", bufs=1, name="qp")
                for cc in range(2):
                    nc.tensor.matmul(
                        out=qp[:],
                        lhsT=wqT_sb[:, cc, :],
                        rhs=xkvt[t][:, cc, :],
                        start=(cc == 0), stop=(cc == 1),
                    )
                nc.vector.tensor_copy(out=q_sb[:, 512 * t:512 * (t + 1)], in_=qp[:])

            def emit_vproj(t):
                # vT[n, inner] = x^T @ Wv^T, 128-row tiles of n
                vp = ps_st.tile([128, 4, 128], f32, tag="acc", bufs=1, name="vp")
                for t2 in range(4):
                    for cc in range(2):
                        nc.tensor.matmul(
                            out=vp[:, t2, :],
                            lhsT=xkvt[t][:, cc, 128 * t2:128 * (t2 + 1)],
                            rhs=wvT_sb[:, cc, :],
                            start=(cc == 0), stop=(cc == 1),
                        )
                src = vp.rearrange("p t (h d) -> p (t h) d", d=DH)
                if t % 2:
                    nc.scalar.copy(out=vT3[:, 16 * t:16 * (t + 1), 0:DH], in_=src)
                else:
                    nc.vector.tensor_copy(
                        out=vT3[:, 16 * t:16 * (t + 1), 0:DH], in_=src
                    )

            # ---- attention ----
            pt_tiles = {}     # (qb, J, p) -> pt AP

            def emit_j(qb, J):
                for p in range(2):
                    st = ps_st.tile([128, 2, QB], f32, tag="st", name="st")
                    for hh in range(2):
                        h = 2 * p + hh
                        nc.tensor.matmul(
                            out=st[:, hh, :],
                            lhsT=k_sb[32 * h:32 * (h + 1), JT * J:JT * (J + 1)],
                            rhs=q_sb[32 * h:32 * (h + 1), QB * qb:QB * (qb + 1)],
                            start=True, stop=True,
                            tile_position=(32 * h, 0),
                        )
                    pt = ptp.tile([128, 2, QB], f16, tag="pt", name="pt")
                    pt_tiles[(qb, J, p)] = pt
                    use_act = _act_assign(qb, J, p)
                    if use_act:
                        nc.scalar.activation(
                            out=pt[:], in_=st[:],
                            func=mybir.ActivationFunctionType.Exp,
                            scale=SCALE,
                        )
                    else:
                        nc.vector.tensor_scalar(
                            out=pt.bitcast(i16)[:], in0=st[:],
                            scalar1=A_EXP, scalar2=B_EXP,
                            op0=mybir.AluOpType.mult, op1=mybir.AluOpType.add,
                        )

            # PV^T group order per acc bank: h-pairs first so each pt pair-
            # tile's last reader comes early and its slot recycles sooner.
            GORDER = [(0, 0), (0, 1), (1, 0), (1, 1), (0, 2), (0, 3), (1, 2), (1, 3)]

            def start_tail(qb):
                # all four accumulators in ONE 2-bank slot: bank b holds
                # q-subchunks (2b, 2b+1); one open group per bank still holds
                acc = ps_st.tile(
                    [128, 2, 2 * HEADS * (DH + 1)], f32,
                    padded_shape=[128, 2, 512], tag="acc", bufs=1, name="acc",
                )
                av = acc.rearrange("p b (i h d) -> p b i h d", i=2, h=HEADS)
                return {"av": av, "prev": [None, None], "step": 0}

            def emit_pv_group(qb, state, bank, s, j_lo=0, j_hi=NJT):
                # (part of) one accumulation group on acc bank `bank`
                av = state["av"]
                ii, h = GORDER[s]
                i = 2 * bank + ii
                p, hh = h // 2, h % 2
                out_ap = av[:, bank, ii, h, :]
                prev = state["prev"][bank]
                for J in range(j_lo, j_hi):
                    mm = nc.tensor.matmul(
                        out=out_ap,
                        lhsT=pt_tiles[(qb, J, p)][:, hh, 128 * i:128 * (i + 1)],
                        rhs=vT3[:, HEADS * J + h, :],
                        start=(J == 0), stop=(J == NJT - 1),
                        skip_group_check=True,
                    )
                    if prev is not None:
                        add_dep_helper(mm.ins, prev.ins, sync=False, reason="pv order")
                    prev = mm
                state["prev"][bank] = prev

            out_v = out_d.rearrange("(cb p) n -> p cb n", cb=2)

            def alloc_op():
                return ps_st.tile([128, 2, 512], f32, tag="acc", bufs=1, name="op")

            def finish_half(qb, an, half, op, bias_eng=0):
                # out projection + bias + store for q-subchunks 2h, 2h+1
                i0 = 2 * half
                rhs = an[:, i0:i0 + 2, :].rearrange("p b q -> p (b q)")
                for cb in range(2):
                    nc.tensor.matmul(
                        out=op[:, cb, 256 * half:256 * (half + 1)],
                        lhsT=woT_sb[:, 128 * cb:128 * (cb + 1)],
                        rhs=rhs,
                        start=True, stop=True,
                    )
                ob = wk.tile([128, 2, 256], f32, tag="ob", name="ob")
                if bias_eng == 1:
                    for cb in range(2):
                        nc.scalar.add(
                            out=ob[:, cb, :],
                            in_=op[:, cb, 256 * half:256 * (half + 1)],
                            add=bias_sb[:, cb:cb + 1],
                        )
                else:
                    nc.vector.tensor_tensor(
                        out=ob[:], in0=op[:, :, 256 * half:256 * (half + 1)],
                        in1=bias_sb.unsqueeze(2).broadcast_to((128, 2, 256)),
                        op=mybir.AluOpType.add,
                    )
                c0 = QB * qb + 256 * half
                eng = nc.scalar if bias_eng == 2 else nc.sync
                eng.dma_start(out=out_v[:, :, c0:c0 + 256], in_=ob[:])

            def norm_bank(qb, state, bank, anT):
                # normalize: an^T[q, i, h, d] = A^T[q,i,h,d] / A^T[q,i,h,32]
                av = state["av"][:, bank]
                rcp = wk.tile([128, 2, 4], f32, tag="rcp", name="rcp")
                nc.vector.reciprocal(out=rcp[:], in_=av[:, :, :, DH])
                nc.vector.tensor_mul(
                    out=anT[:, 2 * bank:2 * bank + 2],
                    in0=av[:, :, :, 0:DH],
                    in1=rcp.unsqueeze(3).broadcast_to((128, 2, 4, DH)),
                )

            def finish_norm(qb, state):
                anT = wk.tile([128, 4, 4, DH], f16, tag="anT", name="anT")
                norm_bank(qb, state, 0, anT)
                norm_bank(qb, state, 1, anT)
                # batched DMA transpose an^T -> an[inner, i, q] (4 blocks)
                an = wk.tile([128, 4, 128], f16, tag="an", name="an")
                nc.sync.dma_start_transpose(
                    out=an[:], in_=anT.rearrange("q i h d -> q (i h d)")
                )
                state["an"] = an

            # ---- main emission ----
            # qb0 carries the JIT projections; tails of qb spread across the
            # first J's of qb+1 (2 PV^T groups per J over J=2..9, finishers
            # at J=10); the last tail runs after the final J-loop.
            tail_state = None
            tail_qb = None
            emit_qproj(0)
            emit_kproj(0)
            for qb in range(NQB):
                for J in range(NJT):
                    if qb == 0:
                        # JIT projections: k tile (J//4 + prefetch), q, v
                        if J % 4 == 2 and J // 4 + 1 < 8:
                            emit_kproj(J // 4 + 1)
                        if J in (11, 19, 27):
                            emit_qproj((J - 3) // 8)
                        if J % 4 == 1:
                            emit_vproj(J // 4)
                    emit_j(qb, J)
                    if qb == NQB - 1:
                        # pre-run the first PV^T group of each acc bank behind
                        # the exp frontier so less of the final tail serializes
                        if J == 21:
                            st8 = start_tail(qb)
                            emit_pv_group(qb, st8, 0, 0, 0, J - 1)
                            emit_pv_group(qb, st8, 1, 0, 0, J - 1)
                        elif J > 21:
                            emit_pv_group(qb, st8, 0, 0, J - 2, J - 1)
                            emit_pv_group(qb, st8, 1, 0, J - 2, J - 1)
                    if qb > 0:
                        if J == 1:
                            tail_state = start_tail(qb - 1)
                            tail_qb = qb - 1
                        elif 2 <= J <= 17:
                            s = J - 2
                            emit_pv_group(tail_qb, tail_state, s % 2, s // 2)
                        elif J == 18:
                            finish_norm(tail_qb, tail_state)
                        elif J == 19:
                            tail_state["op"] = alloc_op()
                            finish_half(tail_qb, tail_state["an"], 0,
                                        tail_state["op"], bias_eng=1)
                        elif J == 20:
                            finish_half(tail_qb, tail_state["an"], 1,
                                        tail_state["op"], bias_eng=1)
                if qb == NQB - 1:
                    # final tail: bank A fully first so its normalize/
                    # transpose/out-proj overlap bank B's PV^T grind
                    anT = wk.tile([128, 4, 4, DH], f16, tag="anT", name="anT")
                    an = wk.tile([128, 4, 128], f16, tag="an", name="an")
                    emit_pv_group(qb, st8, 0, 0, NJT - 2, NJT)
                    emit_pv_group(qb, st8, 1, 0, NJT - 2, NJT)
                    for s in range(1, 8):
                        emit_pv_group(qb, st8, 0, s)
                    norm_bank(qb, st8, 0, anT)
                    nc.scalar.dma_start_transpose(
                        out=an[:, 0:2, :],
                        in_=anT[:, 0:2].rearrange("q i h d -> q (i h d)"),
                    )
                    for s in range(1, 8):
                        emit_pv_group(qb, st8, 1, s)
                    norm_bank(qb, st8, 1, anT)
                    trp = ps_st.tile([128, 2, 128], f16, tag="st", name="trp")
                    for ii in range(2):
                        nc.tensor.transpose(
                            out=trp[:, ii, :],
                            in_=anT[:, 2 + ii].rearrange("q h d -> q (h d)"),
                            identity=eye_sb[:],
                        )
                    nc.scalar.copy(out=an[:, 2:4, :], in_=trp[:])
                    op_end = ps_st.tile([128, 2, 512], f32, tag="st", name="op_end")
                    finish_half(qb, an, 0, op_end, bias_eng=2)
                    finish_half(qb, an, 1, op_end, bias_eng=2)

    nc.compile()
    return nc


_NC_CACHE = []


def _get_nc():
    if not _NC_CACHE:
        _NC_CACHE.append(build_nc())
    return _NC_CACHE[0]


def _make_in_maps(x, Wq, Wk, Wv, Wout, bout):
    bfl = np.float16
    xf = np.asarray(x, dtype=np.float32).reshape(B, C, N)
    wqT = np.ascontiguousarray(np.asarray(Wq, np.float32).T).astype(bfl)
    wkT = np.ascontiguousarray(np.asarray(Wk, np.float32).T).astype(bfl)
    wvT = np.ascontiguousarray(np.asarray(Wv, np.float32).T).astype(bfl)
    woT = np.ascontiguousarray(np.asarray(Wout, np.float32).T).astype(bfl)
    biasT = np.ascontiguousarray(
        np.asarray(bout, np.float32).reshape(2, 128).T
    ).astype(np.float32)
    eye = np.eye(128, dtype=np.float16)
    in_maps = []
    for core in range(8):
        b, half = core // 2, core % 2
        q0 = half * NQ
        # roll keys so this core's queries occupy columns 0:NQ; key order is
        # shared by k and v so softmax/PV are unaffected.
        xroll = np.roll(xf[b], -q0, axis=1) if q0 else xf[b]
        in_maps.append({
            "xkv": np.ascontiguousarray(xroll).astype(bfl),
            "wqT": wqT, "wkT": wkT, "wvT": wvT,
            "woT": woT, "biasT": biasT, "eye": eye,
        })
    return in_maps


def kernel(x, Wq, Wk, Wv, Wout, bout):
    nc = _get_nc()
    in_maps = _make_in_maps(x, Wq, Wk, Wv, Wout, bout)
    res = run_bass_kernel_spmd(nc, in_maps, core_ids=list(range(8)))
    out = np.empty((B, C, N), dtype=np.float32)
    for core in range(8):
        b, half = core // 2, core % 2
        q0 = half * NQ
        out[b][:, q0:q0 + NQ] = res.results[core]["out"]
    return out.reshape(B, C, 64, 64)
